# revision 16
# baseline (speedup 1.0000x reference)
"""Multi-head attention (B=8, N=2048, C=320, H=5, D=64) on 8 Trainium2 cores.

Sharding: data-parallel over batch — core b computes attention for x[b].
Weights are replicated. No collectives.

Per-core strategy (fp16 operands, fp32 accumulation):
  - Every matmul in the steady state uses the full (128,128) PE tile config
    so LDWEIGHTS always overlaps the previous matmul's streaming (a config
    switch costs ~90ns of unoverlapped weight load).  Scores contract over
    128 partitions with the per-head k^T stationary zero-padded outside the
    head's 64-channel band (kTp); the moving qT plane rows belonging to the
    sibling head are killed by the zero rows.
  - w_qkv^T is stored with each of q/k padded to 384 columns (zero columns)
    so all projection stationaries are 128 wide.
  - Scores computed transposed: S^T[m, n] = sum_d kTp[d,m] qT[d,n]; exp on
    the scalar engine produces P^T directly as the AV moving operand with
    lhsT = [V_h | ones]; row 64 of the AV output is the softmax denominator.
  - AV eviction is split so no engine queue head-of-line blocks: an
    immediate unscaled fp16 copy frees the PSUM tile, then the reciprocal
    chain (denom row -> DRAM -> [128,8] scatter -> DVE reciprocal -> row ->
    GPSIMD partition-broadcast) runs as deferred jobs inside the next
    phase, and GPSIMD (not DVE) applies the 1/denom scale into oTp.
  - The projection is a clean 5-matmul PSUM accumulation over heads plus
    one bias add; proj(chunk c) is interleaved two phases after the last
    head of chunk c so the reciprocal chain is never on the critical path.
  - PSUM: scores pool 2x[128,1024] + output pool 2x[65,1024] = 8 banks; the
    double-buffered output tile removes the per-head eviction stall.
  - fp32->fp16 input casts run on the scalar engine (idle during startup);
    startup PE transposes rotate across both PSUM pools.
"""

import numpy as np

import concourse.bacc as bacc
import concourse.tile as tile
from concourse import mybir
from concourse.bass_utils import run_bass_kernel_spmd
from concourse.masks import make_identity

FP32 = mybir.dt.float32
FP16 = mybir.dt.float16
AF = mybir.ActivationFunctionType
ALU = mybir.AluOpType

B = 8
C = 320
H = 5
D = 64
SCALE = D ** -0.5
# c-dim tiles (contraction tiles for the C=320 dim); all matmuls use the
# full 128 partitions — rows 64..127 of the third tile are zeroed.
CT = [(0, 128), (128, 128), (256, 64)]
# padded w^T column sections: q @ 0, k @ 384, v @ 768 (each q/k padded to 384)
QSEC, KSEC, VSEC = 0, 384, 768
WCOLS = 1088


def build_program(N: int):
    """Build + compile the single-core Bass program (SPMD across 8 cores)."""
    nc = bacc.Bacc("TRN2", target_bir_lowering=False, debug=False)

    x_d = nc.dram_tensor("x", [N, C], FP32, kind="ExternalInput")
    wqkv_d = nc.dram_tensor("w_qkv", [3 * C, C], FP32, kind="ExternalInput")
    wproj_d = nc.dram_tensor("w_proj", [C, C], FP32, kind="ExternalInput")
    bproj_d = nc.dram_tensor("b_proj", [C], FP32, kind="ExternalInput")
    out_d = nc.dram_tensor("out", [N, C], FP32, kind="ExternalOutput")

    MT = N // 128                       # number of 128-row seq tiles
    CHUNK = 1024 if N % 1024 == 0 else N
    NCH = N // CHUNK                    # attention n-chunks
    PT_CH = CHUNK // 128                # proj n-tiles per chunk

    with tile.TileContext(nc) as tc:
        with (
            tc.tile_pool(name="persist", bufs=1) as per,
            tc.tile_pool(name="ld", bufs=3) as ld,
            tc.tile_pool(name="s_ps", bufs=2, space="PSUM") as s_ps,
            tc.tile_pool(name="o_ps", bufs=2, space="PSUM") as o_ps,
            tc.tile_pool(name="pt", bufs=4) as pt_pool,
            tc.tile_pool(name="misc", bufs=2) as misc,
            tc.tile_pool(name="yacc", bufs=3) as yacc_pool,
        ):
            identity = per.tile([128, 128], FP32)
            make_identity(nc, identity[:])
            identity_h = per.tile([128, 128], FP16)
            nc.vector.tensor_copy(identity_h[:], identity[:])

            wT = per.tile([128, 3, WCOLS], FP16)   # w_qkv^T, padded sections
            xT = per.tile([128, 3, N], FP16)
            qT = per.tile([128, 3, N], FP16)
            kTp = per.tile([128, H, N], FP16)      # per-head k^T, zero-padded
            VE = 128  # per-head AV stationary: [ones | 63 pad | V(64)]
            v_sb = per.tile([128, MT, H * VE], FP16)
            oTp = per.tile([128, H, N], FP16)      # rows 0-63: O^T_h scaled
            wpt = per.tile([128, H, C], FP16)      # w_proj^T per head (top 64)
            bias_sb = per.tile([128, C], FP32)
            b_row = per.tile([1, C], FP32)
            ones1 = per.tile([1, 128], FP32)

            # ---- zero padding memsets ----
            # kTp band for head h occupies partitions 64*(h%2)..; the
            # complement must be zero (it is the scores stationary).
            v_heads = v_sb[:].rearrange("p m (h e) -> p m h e", h=H)
            nc.gpsimd.memset(v_heads[:, :, :, 0:1], 1.0)
            nc.gpsimd.memset(v_heads[:, :, :, 1:64], 0.0)
            nc.gpsimd.memset(xT[64:128, 2, :], 0.0)
            nc.gpsimd.memset(wT[64:128, 2, :], 0.0)
            for h in range(2):
                off = 64 * (h % 2)
                nc.gpsimd.memset(kTp[64 - off : 128 - off, h, :], 0.0)
            nc.vector.memset(ones1[:], 1.0)
            nc.vector.memset(wT[:, :, QSEC + 320 : QSEC + 384], 0.0)
            nc.vector.memset(wT[:, :, KSEC + 320 : KSEC + 384], 0.0)
            for h in range(2, H):
                off = 64 * (h % 2)
                nc.gpsimd.memset(kTp[64 - off : 128 - off, h, :], 0.0)
            nc.gpsimd.memset(oTp[0:64, :, :], 0.0)

            # startup transposes rotate across both PSUM pools (o_ps is idle
            # until the first attention phase)
            tp_state = [0]

            def transpose_fp16(dst_ap, src_ap, rp, cp, startup=False):
                """dst[cp, rp] = src[rp, cp].T via PE transpose (fp16)."""
                if startup and tp_state[0] % 2:
                    ps = o_ps.tile([128, 512], FP16, tag="ot")
                else:
                    ps = s_ps.tile([128, 512], FP16, tag="s")
                tp_state[0] += 1
                nc.tensor.transpose(ps[:cp, :rp], src_ap, identity_h[:rp, :rp])
                nc.vector.tensor_copy(dst_ap, ps[:cp, :rp])

            # ---- w_qkv -> wT (w_qkv^T into padded sections) ----
            def wcol(r):
                """padded wT column for w_qkv row r."""
                if r < 320:
                    return QSEC + r
                if r < 640:
                    return KSEC + (r - 320)
                return VSEC + (r - 640)

            def emit_w_tile(wt, startup=True):
                r0 = wt * 128
                rp = min(128, 3 * C - r0)
                wnat = ld.tile([128, C], FP32, tag="wnat")
                nc.sync.dma_start(wnat[:rp, :], wqkv_d.ap()[r0 : r0 + rp, :])
                wnat_h = ld.tile([128, C], FP16, tag="wnat_h")
                nc.scalar.activation(wnat_h[:rp, :], wnat[:rp, :], AF.Copy)
                splits = [r0]
                for bnd in (320, 640):
                    if r0 < bnd < r0 + rp:
                        splits.append(bnd)
                splits.append(r0 + rp)
                for ci, (c0, cp) in enumerate(CT):
                    if startup and tp_state[0] % 2:
                        ps = o_ps.tile([128, 512], FP16, tag="ot")
                    else:
                        ps = s_ps.tile([128, 512], FP16, tag="s")
                    tp_state[0] += 1
                    nc.tensor.transpose(
                        ps[:cp, :rp],
                        wnat_h[:rp, c0 : c0 + cp],
                        identity_h[:rp, :rp],
                    )
                    for a, b_ in zip(splits, splits[1:]):
                        d0 = wcol(a)
                        nc.vector.tensor_copy(
                            wT[:cp, ci, d0 : d0 + (b_ - a)],
                            ps[:cp, a - r0 : b_ - r0],
                        )

            # ---- x -> xT (DMA, ACT cast, PE transpose) + q0/k0 chunks ----
            # qk section s (0..5 = q0,q1,q2p,k0,k1,k2p): 128 padded channels.
            def emit_qk_chunk(sec, g):
                base = (QSEC if sec < 3 else KSEC) + 128 * (sec % 3)
                s0 = g * 512
                sw = min(512, N - s0)
                ps = s_ps.tile([128, 512], FP32, tag="s")
                for ci in range(3):
                    nc.tensor.matmul(
                        ps[:, :sw],
                        wT[:, ci, base : base + 128],
                        xT[:, ci, s0 : s0 + sw],
                        start=(ci == 0),
                        stop=(ci == 2),
                    )
                if sec < 3:
                    # q plane j=sec; full 128 rows (zero rows land in plane 2)
                    nc.vector.tensor_copy(qT[:, sec, s0 : s0 + sw], ps[:, :sw])
                else:
                    j = sec - 3
                    h0 = 2 * j
                    nc.vector.tensor_copy(
                        kTp[0:64, h0, s0 : s0 + sw], ps[0:64, :sw]
                    )
                    if h0 + 1 < H:
                        nc.vector.tensor_copy(
                            kTp[64:128, h0 + 1, s0 : s0 + sw], ps[64:128, :sw]
                        )

            # DMA + cast ALL x groups up front (4-deep ld buffers: the DMA
            # stream never waits on recycling); per-group casts run on the
            # scalar engine which is idle before the first exp.
            x_re = x_d.ap().rearrange("(t p) c -> p t c", p=128)
            NG = (MT + 3) // 4
            xh_tiles = []
            for g in range(NG):
                gn = min(4, MT - 4 * g)
                xnat = ld.tile([128, 4, C], FP32, tag="xnat", bufs=NG)
                nc.sync.dma_start(xnat[:, :gn, :], x_re[:, 4 * g : 4 * g + gn, :])
                xnat_h = ld.tile([128, 4, C], FP16, tag="xnat_h", bufs=NG)
                nc.scalar.activation(xnat_h[:, :gn, :], xnat[:, :gn, :], AF.Copy)
                xh_tiles.append(xnat_h)

            for wt in (0, 2, 3):
                emit_w_tile(wt)

            def emit_x_transpose(mt):
                xnat_h = xh_tiles[mt // 4]
                t = mt % 4
                for ci, (c0, cp) in enumerate(CT):
                    transpose_fp16(
                        xT[:cp, ci, mt * 128 : (mt + 1) * 128],
                        xnat_h[:, t, c0 : c0 + cp],
                        128,
                        cp,
                        startup=True,
                    )

            # prologue PE work: only what the first scores iteration needs —
            # x groups 0-1, q0 over cols 0-1023, k0 over m-tiles 0-3, and the
            # v section weights (v tiles start at the first attention
            # iteration).  The rest defers into attention phases as jobs.
            for mt in range(8):
                emit_x_transpose(mt)
            emit_qk_chunk(0, 0)
            emit_qk_chunk(0, 1)
            emit_qk_chunk(3, 0)
            for wt in (5, 6, 7):
                emit_w_tile(wt)
            emit_w_tile(4)
            emit_w_tile(1)

            # ---- bias broadcast [128, C] ----
            nc.sync.dma_start(b_row[:], bproj_d.ap().rearrange("(a c) -> a c", a=1))
            ps = s_ps.tile([128, 512], FP32, tag="s")
            nc.tensor.matmul(ps[:, :C], ones1[:], b_row[:], start=True, stop=True)
            nc.vector.tensor_copy(bias_sb[:], ps[:, :C])

            # ---- interleavable jobs ----
            def emit_v_tile(mt):
                ps = s_ps.tile([128, 512], FP32, tag="s")
                for ci in range(3):
                    nc.tensor.matmul(
                        ps[:, :C],
                        xT[:, ci, mt * 128 : (mt + 1) * 128],
                        wT[:, ci, VSEC : VSEC + C],
                        start=(ci == 0),
                        stop=(ci == 2),
                    )
                nc.vector.tensor_copy(
                    v_heads[:, mt, :, 64 : 64 + D],
                    ps[:, :C].rearrange("p (h e) -> p h e", h=H),
                )

            def emit_wproj_tile(wt):
                r0, rp = CT[wt]
                wpnat = ld.tile([128, C], FP32, tag="wnat")
                nc.sync.dma_start(wpnat[:rp, :], wproj_d.ap()[r0 : r0 + rp, :])
                # 64 pad columns in front: transposing [hD-64 .. hD+63]
                # puts w_proj channel d at wpt row 64+d, matching the AV
                # output rows (rows <64 hit pad/other-head junk, which the
                # zero rows 0..63 of oTp kill in the proj matmul).
                wpnat_h = ld.tile([128, 64 + C], FP16, tag="wpnat_h")
                nc.vector.memset(wpnat_h[:, 0:64], 0.0)
                nc.scalar.activation(
                    wpnat_h[:rp, 64 : 64 + C], wpnat[:rp, :], AF.Copy
                )
                for h in range(H):
                    transpose_fp16(
                        wpt[0:VE, h, r0 : r0 + rp],
                        wpnat_h[:rp, h * D : h * D + VE],
                        rp,
                        VE,
                    )

            def finish_proj(gt, yp):
                nc.tensor.matmul(
                    yp[:, :C],
                    oTp[:, H - 1, gt * 128 : (gt + 1) * 128],
                    wpt[:, H - 1, :],
                    start=False,
                    stop=True,
                )
                acc = yacc_pool.tile([128, C], FP32, tag="acc")
                nc.vector.tensor_tensor(acc[:], yp[:, :C], bias_sb[:], ALU.add)
                nc.sync.dma_start(out_d.ap()[gt * 128 : (gt + 1) * 128, :], acc[:])

            def emit_proj_tile(gt):
                yp = s_ps.tile([128, CHUNK], FP32, tag="s")
                for h in range(H):
                    nc.tensor.matmul(
                        yp[:, :C],
                        oTp[:, h, gt * 128 : (gt + 1) * 128],
                        wpt[:, h, :],
                        start=(h == 0),
                        stop=(h == H - 1),
                    )
                acc = yacc_pool.tile([128, C], FP32, tag="acc")
                nc.vector.tensor_tensor(acc[:], yp[:, :C], bias_sb[:], ALU.add)
                nc.sync.dma_start(out_d.ap()[gt * 128 : (gt + 1) * 128, :], acc[:])

            # ---- attention ----
            from collections import deque

            jobq = deque()
            chainq = deque()
            av_backlog = deque()

            def emit_attention(h, nci, pops=1):
                n0 = nci * CHUNK
                jt = h // 2
                ot_ps = o_ps.tile([128, CHUNK], FP32, tag="ot")

                def make_av(mt, pt):
                    def f():
                        for s0 in range(0, CHUNK, 512):
                            nc.tensor.matmul(
                                ot_ps[:, s0 : s0 + 512],
                                v_sb[:, mt, h * VE : (h + 1) * VE],
                                pt[:, s0 : s0 + 512],
                                start=(mt == 0),
                                stop=(mt == MT - 1),
                            )
                    return f

                # depth-2 software pipeline: AV(mt) is emitted two iterations
                # after scores(mt), giving the scalar engine two full
                # iterations to produce exp(mt) — AV never head-of-line
                # blocks the in-order PE queue on the activation.
                for mt in range(MT):
                    if mt >= 2 and chainq:
                        chainq.popleft()()
                    for _ in range(pops):
                        if jobq:
                            jobq.popleft()()
                    sp = s_ps.tile([128, CHUNK], FP32, tag="s")
                    for s0 in range(0, CHUNK, 512):
                        nc.tensor.matmul(
                            sp[:, s0 : s0 + 512],
                            kTp[:, h, mt * 128 : (mt + 1) * 128],
                            qT[:, jt, n0 + s0 : n0 + s0 + 512],
                            start=True,
                            stop=True,
                        )
                    pt = pt_pool.tile([128, CHUNK], FP16, tag="pt")
                    nc.scalar.activation(pt[:], sp[:], AF.Exp, scale=SCALE)
                    av_backlog.append(make_av(mt, pt))
                    while len(av_backlog) > 2:
                        av_backlog.popleft()()

                # eviction + reciprocal chain, deferred into the next phase.
                # The last two AVs of this phase drain from av_backlog during
                # the next phase's first two iterations, so these jobs are
                # inserted at queue position >= 2 (after padding) to keep
                # emission order legal (they read ot_ps after AV(15)).
                box = []

                def job_evict():
                    # unscaled O^T (rows 64..127) + denominator row 0; the
                    # denominator rides row 0 of the AV output because
                    # partition_broadcast only reads physical partition 0.
                    oraw = misc.tile([128, CHUNK], FP16, tag="oraw")
                    nc.vector.tensor_copy(oraw[64:128, :], ot_ps[64:128, :])
                    dstage = misc.tile([1, CHUNK], FP32, tag="dst")
                    nc.vector.tensor_copy(dstage[0:1, :], ot_ps[0:1, :])
                    box.append(oraw)
                    box.append(dstage)

                def job_bcast():
                    denomB = misc.tile([128, CHUNK], FP32, tag="denomB")
                    nc.gpsimd.partition_broadcast(
                        denomB[:], box[1][0:1, :], channels=128
                    )
                    box.append(denomB)

                def job_recip():
                    recipB = misc.tile([128, CHUNK], FP32, tag="recipB")
                    nc.vector.reciprocal_approx_fast(recipB[:], box[2][:])
                    box.append(recipB)

                def job_mult():
                    nc.vector.tensor_tensor(
                        oTp[64:128, h, n0 : n0 + CHUNK],
                        box[0][64:128, :],
                        box[3][64:128, :],
                        ALU.mult,
                    )

                chainq.extend((job_evict, job_bcast, job_recip, job_mult))

            # job schedule.  (0,0) carries the deferred startup work (x
            # transposes for groups 2-3, k0 chunks 1-3, all v tiles) at two
            # pops per iteration; later phases spread the remaining qkv
            # projections, w_proj load, and the output projection so the PE
            # load per phase stays near the exp cadence.  k-plane chunks are
            # scheduled before the m-tiles that consume them; q-plane
            # 1024.. columns are produced before the chunk-1 phases.
            x_jobs = {mt: (lambda m=mt: emit_x_transpose(m)) for mt in range(8, MT)}
            v_job = {mt: (lambda m=mt: emit_v_tile(m)) for mt in range(MT)}
            qk_job = {(s, g): (lambda ss=s, gg=g: emit_qk_chunk(ss, gg))
                      for s in range(6) for g in range(N // 512)}
            wp_jobs = [lambda w=w: emit_wproj_tile(w) for w in range(3)]
            phase00 = [
                qk_job[3, 1], v_job[0],
                v_job[1], x_jobs[8],
                v_job[2], x_jobs[9],
                v_job[3], x_jobs[10],
                v_job[4], x_jobs[11],
                v_job[5], qk_job[3, 2],
                v_job[6], x_jobs[12],
                v_job[7], x_jobs[13],
                v_job[8], x_jobs[14],
                v_job[9], x_jobs[15],
                v_job[10], qk_job[3, 3],
            ] + [v_job[mt] for mt in range(11, MT)]
            phase_jobs = {
                (0, 0): phase00,
                (0, 1): [qk_job[1, 0], qk_job[1, 1], qk_job[4, 0]],
                (0, 2): [qk_job[4, 1], qk_job[4, 2], qk_job[4, 3],
                         qk_job[2, 0], qk_job[2, 1], qk_job[5, 0]],
                (0, 3): [qk_job[5, 1], qk_job[5, 2], qk_job[5, 3]] + wp_jobs,
                (0, 4): [qk_job[0, 2], qk_job[0, 3]],
                (1, 1): [lambda g=t: emit_proj_tile(g) for t in range(4)]
                        + [qk_job[1, 2], qk_job[1, 3]],
                (1, 2): [lambda g=t: emit_proj_tile(g) for t in range(4, PT_CH)],
                (1, 3): [qk_job[2, 2], qk_job[2, 3]],
            }
            for nci in range(NCH):
                for h in range(H):
                    jobq.extend(phase_jobs.get((nci, h), []))
                    emit_attention(h, nci, pops=2 if (nci, h) == (0, 0) else 1)
            while av_backlog:
                av_backlog.popleft()()
            while chainq:
                chainq.popleft()()
            while jobq:
                jobq.popleft()()
            # tail projection, pipelined in two stages: each tile's first
            # four head-accumulation matmuls don't depend on the final
            # head's reciprocal chain, so they run while it completes.
            pend = None
            for t in range(PT_CH):
                gt = (NCH - 1) * PT_CH + t
                yp = s_ps.tile([128, CHUNK], FP32, tag="s")
                for h in range(H - 1):
                    nc.tensor.matmul(
                        yp[:, :C],
                        oTp[:, h, gt * 128 : (gt + 1) * 128],
                        wpt[:, h, :],
                        start=(h == 0),
                        stop=False,
                    )
                if pend is not None:
                    finish_proj(*pend)
                pend = (gt, yp)
            finish_proj(*pend)

    nc.compile()
    return nc


_cache = {}


def _get_program(N: int):
    if N not in _cache:
        _cache[N] = build_program(N)
    return _cache[N]


def kernel(x, w_qkv, w_proj, b_proj):
    x = np.ascontiguousarray(np.asarray(x, dtype=np.float32))
    w_qkv = np.ascontiguousarray(np.asarray(w_qkv, dtype=np.float32))
    w_proj = np.ascontiguousarray(np.asarray(w_proj, dtype=np.float32))
    b_proj = np.ascontiguousarray(np.asarray(b_proj, dtype=np.float32))
    Bx, N, Cx = x.shape
    assert Bx == B and Cx == C, (x.shape,)

    nc = _get_program(N)
    in_maps = [
        {"x": x[b], "w_qkv": w_qkv, "w_proj": w_proj, "b_proj": b_proj}
        for b in range(B)
    ]
    res = run_bass_kernel_spmd(nc, in_maps, core_ids=list(range(B)))
    return np.stack([res.results[b]["out"] for b in range(B)], axis=0)


# revision 17
# speedup vs baseline: 1.2136x; 1.2136x over previous
"""Multi-head attention (B=8, N=2048, C=320, H=5, D=64) on 8 Trainium2 cores.

Sharding: data-parallel over batch — core b computes attention for x[b].
Weights are replicated. No collectives.

Per-core strategy (fp16 operands, fp32 accumulation):
  - Every matmul in the steady state uses the full (128,128) PE tile config
    so LDWEIGHTS always overlaps the previous matmul's streaming (a config
    switch costs ~90ns of unoverlapped weight load).  Scores contract over
    128 partitions with the per-head k^T stationary zero-padded outside the
    head's 64-channel band (kTp); the moving qT plane rows belonging to the
    sibling head are killed by the zero rows.
  - w_qkv^T is stored with each of q/k padded to 384 columns (zero columns)
    so all projection stationaries are 128 wide.
  - Scores computed transposed: S^T[m, n] = sum_d kTp[d,m] qT[d,n]; exp on
    the scalar engine produces P^T directly as the AV moving operand with
    lhsT = [V_h | ones]; row 64 of the AV output is the softmax denominator.
  - AV eviction is split so no engine queue head-of-line blocks: an
    immediate unscaled fp16 copy frees the PSUM tile, then the reciprocal
    chain (denom row -> DRAM -> [128,8] scatter -> DVE reciprocal -> row ->
    GPSIMD partition-broadcast) runs as deferred jobs inside the next
    phase, and GPSIMD (not DVE) applies the 1/denom scale into oTp.
  - The projection is a clean 5-matmul PSUM accumulation over heads plus
    one bias add; proj(chunk c) is interleaved two phases after the last
    head of chunk c so the reciprocal chain is never on the critical path.
  - PSUM: scores pool 2x[128,1024] + output pool 2x[65,1024] = 8 banks; the
    double-buffered output tile removes the per-head eviction stall.
  - fp32->fp16 input casts run on the scalar engine (idle during startup);
    startup PE transposes rotate across both PSUM pools.
"""

import numpy as np

import concourse.bacc as bacc
import concourse.tile as tile
from concourse import mybir
from concourse.bass_utils import run_bass_kernel_spmd
from concourse.masks import make_identity

FP32 = mybir.dt.float32
FP16 = mybir.dt.float16
AF = mybir.ActivationFunctionType
ALU = mybir.AluOpType

B = 8
C = 320
H = 5
D = 64
SCALE = D ** -0.5
# c-dim tiles (contraction tiles for the C=320 dim); all matmuls use the
# full 128 partitions — rows 64..127 of the third tile are zeroed.
CT = [(0, 128), (128, 128), (256, 64)]
# padded w^T column sections: q @ 0, k @ 384, v @ 768 (each q/k padded to 384)
QSEC, KSEC, VSEC = 0, 384, 768
WCOLS = 1088


def build_program(N: int):
    """Build + compile the single-core Bass program (SPMD across 8 cores)."""
    nc = bacc.Bacc("TRN2", target_bir_lowering=False, debug=False)

    x_d = nc.dram_tensor("x", [N, C], FP32, kind="ExternalInput")
    wqkv_d = nc.dram_tensor("w_qkv", [3 * C, C], FP32, kind="ExternalInput")
    wproj_d = nc.dram_tensor("w_proj", [C, C], FP32, kind="ExternalInput")
    bproj_d = nc.dram_tensor("b_proj", [C], FP32, kind="ExternalInput")
    out_d = nc.dram_tensor("out", [N, C], FP32, kind="ExternalOutput")

    MT = N // 128                       # number of 128-row seq tiles
    CHUNK = 1024 if N % 1024 == 0 else N
    NCH = N // CHUNK                    # attention n-chunks
    PT_CH = CHUNK // 128                # proj n-tiles per chunk

    with tile.TileContext(nc) as tc:
        with (
            tc.tile_pool(name="persist", bufs=1) as per,
            tc.tile_pool(name="ld", bufs=3) as ld,
            tc.tile_pool(name="s_ps", bufs=2, space="PSUM") as s_ps,
            tc.tile_pool(name="o_ps", bufs=2, space="PSUM") as o_ps,
            tc.tile_pool(name="pt", bufs=4) as pt_pool,
            tc.tile_pool(name="misc", bufs=2) as misc,
            tc.tile_pool(name="yacc", bufs=3) as yacc_pool,
        ):
            identity = per.tile([128, 128], FP32)
            make_identity(nc, identity[:])
            identity_h = per.tile([128, 128], FP16)
            nc.vector.tensor_copy(identity_h[:], identity[:])

            wT = per.tile([128, 3, WCOLS], FP16)   # w_qkv^T, padded sections
            xT = per.tile([128, 3, N], FP16)
            qT = per.tile([128, 3, N], FP16)
            kTp = per.tile([128, H, N], FP16)      # per-head k^T, zero-padded
            VE = 128  # per-head AV stationary: [ones | 63 pad | V(64)]
            v_sb = per.tile([128, MT, H * VE], FP16)
            oTp = per.tile([128, H, N], FP16)      # rows 0-63: O^T_h scaled
            wpt = per.tile([128, H, C], FP16)      # w_proj^T per head (top 64)
            bias_sb = per.tile([128, C], FP32)
            b_row = per.tile([1, C], FP32)
            ones1 = per.tile([1, 128], FP32)

            # ---- zero padding memsets ----
            # kTp band for head h occupies partitions 64*(h%2)..; the
            # complement must be zero (it is the scores stationary).
            v_heads = v_sb[:].rearrange("p m (h e) -> p m h e", h=H)
            nc.gpsimd.memset(v_heads[:, :, :, 0:1], 1.0)
            nc.gpsimd.memset(v_heads[:, :, :, 1:64], 0.0)
            nc.gpsimd.memset(xT[64:128, 2, :], 0.0)
            nc.gpsimd.memset(wT[64:128, 2, :], 0.0)
            for h in range(2):
                off = 64 * (h % 2)
                nc.gpsimd.memset(kTp[64 - off : 128 - off, h, :], 0.0)
            nc.vector.memset(ones1[:], 1.0)
            nc.vector.memset(wT[:, :, QSEC + 320 : QSEC + 384], 0.0)
            nc.vector.memset(wT[:, :, KSEC + 320 : KSEC + 384], 0.0)
            for h in range(2, H):
                off = 64 * (h % 2)
                nc.gpsimd.memset(kTp[64 - off : 128 - off, h, :], 0.0)
            nc.gpsimd.memset(oTp[0:64, :, :], 0.0)

            # startup transposes rotate across both PSUM pools (o_ps is idle
            # until the first attention phase)
            tp_state = [0]

            def transpose_fp16(dst_ap, src_ap, rp, cp, startup=False):
                """dst[cp, rp] = src[rp, cp].T via PE transpose (fp16)."""
                if startup and tp_state[0] % 2:
                    ps = o_ps.tile([128, 512], FP16, tag="ot")
                else:
                    ps = s_ps.tile([128, 512], FP16, tag="s")
                tp_state[0] += 1
                nc.tensor.transpose(ps[:cp, :rp], src_ap, identity_h[:rp, :rp])
                nc.vector.tensor_copy(dst_ap, ps[:cp, :rp])

            # ---- w_qkv -> wT (w_qkv^T into padded sections) ----
            def wcol(r):
                """padded wT column for w_qkv row r."""
                if r < 320:
                    return QSEC + r
                if r < 640:
                    return KSEC + (r - 320)
                return VSEC + (r - 640)

            def emit_w_tile(wt, startup=True):
                r0 = wt * 128
                rp = min(128, 3 * C - r0)
                wnat = ld.tile([128, C], FP32, tag="wnat")
                nc.sync.dma_start(wnat[:rp, :], wqkv_d.ap()[r0 : r0 + rp, :])
                wnat_h = ld.tile([128, C], FP16, tag="wnat_h")
                nc.scalar.activation(wnat_h[:rp, :], wnat[:rp, :], AF.Copy)
                splits = [r0]
                for bnd in (320, 640):
                    if r0 < bnd < r0 + rp:
                        splits.append(bnd)
                splits.append(r0 + rp)
                for ci, (c0, cp) in enumerate(CT):
                    if startup and tp_state[0] % 2:
                        ps = o_ps.tile([128, 512], FP16, tag="ot")
                    else:
                        ps = s_ps.tile([128, 512], FP16, tag="s")
                    tp_state[0] += 1
                    nc.tensor.transpose(
                        ps[:cp, :rp],
                        wnat_h[:rp, c0 : c0 + cp],
                        identity_h[:rp, :rp],
                    )
                    for a, b_ in zip(splits, splits[1:]):
                        d0 = wcol(a)
                        nc.vector.tensor_copy(
                            wT[:cp, ci, d0 : d0 + (b_ - a)],
                            ps[:cp, a - r0 : b_ - r0],
                        )

            # ---- x -> xT (DMA, ACT cast, PE transpose) + q0/k0 chunks ----
            # qk section s (0..5 = q0,q1,q2p,k0,k1,k2p): 128 padded channels.
            def emit_qk_chunk(sec, g):
                base = (QSEC if sec < 3 else KSEC) + 128 * (sec % 3)
                s0 = g * 512
                sw = min(512, N - s0)
                ps = s_ps.tile([128, 512], FP32, tag="s")
                for ci in range(3):
                    nc.tensor.matmul(
                        ps[:, :sw],
                        wT[:, ci, base : base + 128],
                        xT[:, ci, s0 : s0 + sw],
                        start=(ci == 0),
                        stop=(ci == 2),
                    )
                if sec < 3:
                    # q plane j=sec; full 128 rows (zero rows land in plane 2)
                    nc.vector.tensor_copy(qT[:, sec, s0 : s0 + sw], ps[:, :sw])
                else:
                    j = sec - 3
                    h0 = 2 * j
                    nc.vector.tensor_copy(
                        kTp[0:64, h0, s0 : s0 + sw], ps[0:64, :sw]
                    )
                    if h0 + 1 < H:
                        nc.vector.tensor_copy(
                            kTp[64:128, h0 + 1, s0 : s0 + sw], ps[64:128, :sw]
                        )

            # DMA + cast ALL x groups up front (4-deep ld buffers: the DMA
            # stream never waits on recycling); per-group casts run on the
            # scalar engine which is idle before the first exp.
            x_re = x_d.ap().rearrange("(t p) c -> p t c", p=128)
            NG = (MT + 3) // 4
            xh_tiles = []

            def emit_x_group(g):
                gn = min(4, MT - 4 * g)
                xnat = ld.tile([128, 4, C], FP32, tag="xnat", bufs=NG)
                nc.sync.dma_start(xnat[:, :gn, :], x_re[:, 4 * g : 4 * g + gn, :])
                xnat_h = ld.tile([128, 4, C], FP16, tag="xnat_h", bufs=NG)
                nc.scalar.activation(xnat_h[:, :gn, :], xnat[:, :gn, :], AF.Copy)
                xh_tiles.append(xnat_h)

            # interleave the w and x loads so neither the transposes (need
            # x early) nor the q0/k0 projections (need w0/2/3) starve.
            emit_w_tile(0)
            emit_x_group(0)
            emit_w_tile(2)
            emit_w_tile(3)
            emit_x_group(1)

            def emit_x_transpose(mt):
                xnat_h = xh_tiles[mt // 4]
                t = mt % 4
                for ci, (c0, cp) in enumerate(CT):
                    transpose_fp16(
                        xT[:cp, ci, mt * 128 : (mt + 1) * 128],
                        xnat_h[:, t, c0 : c0 + cp],
                        128,
                        cp,
                        startup=True,
                    )

            # prologue PE work: only what the first scores iteration needs —
            # x groups 0-1, q0 over cols 0-1023, k0 over m-tiles 0-3, and the
            # v section weights (v tiles start at the first attention
            # iteration).  The rest defers into attention phases as jobs.
            for mt in range(4):
                emit_x_transpose(mt)
            emit_x_group(2)
            emit_w_tile(5)
            for mt in range(4, 8):
                emit_x_transpose(mt)
            emit_x_group(3)
            emit_w_tile(6)
            emit_w_tile(7)
            emit_qk_chunk(0, 0)
            emit_qk_chunk(0, 1)
            emit_qk_chunk(3, 0)
            emit_w_tile(4)
            emit_w_tile(1)

            # ---- bias broadcast [128, C] ----
            nc.sync.dma_start(b_row[:], bproj_d.ap().rearrange("(a c) -> a c", a=1))
            ps = s_ps.tile([128, 512], FP32, tag="s")
            nc.tensor.matmul(ps[:, :C], ones1[:], b_row[:], start=True, stop=True)
            nc.vector.tensor_copy(bias_sb[:], ps[:, :C])

            # ---- interleavable jobs ----
            def emit_v_tile(mt):
                ps = s_ps.tile([128, 512], FP32, tag="s")
                for ci in range(3):
                    nc.tensor.matmul(
                        ps[:, :C],
                        xT[:, ci, mt * 128 : (mt + 1) * 128],
                        wT[:, ci, VSEC : VSEC + C],
                        start=(ci == 0),
                        stop=(ci == 2),
                    )
                nc.vector.tensor_copy(
                    v_heads[:, mt, :, 64 : 64 + D],
                    ps[:, :C].rearrange("p (h e) -> p h e", h=H),
                )

            def emit_wproj_tile(wt):
                r0, rp = CT[wt]
                wpnat = ld.tile([128, C], FP32, tag="wnat")
                nc.sync.dma_start(wpnat[:rp, :], wproj_d.ap()[r0 : r0 + rp, :])
                # 64 pad columns in front: transposing [hD-64 .. hD+63]
                # puts w_proj channel d at wpt row 64+d, matching the AV
                # output rows (rows <64 hit pad/other-head junk, which the
                # zero rows 0..63 of oTp kill in the proj matmul).
                wpnat_h = ld.tile([128, 64 + C], FP16, tag="wpnat_h")
                nc.vector.memset(wpnat_h[:, 0:64], 0.0)
                nc.scalar.activation(
                    wpnat_h[:rp, 64 : 64 + C], wpnat[:rp, :], AF.Copy
                )
                for h in range(H):
                    transpose_fp16(
                        wpt[0:VE, h, r0 : r0 + rp],
                        wpnat_h[:rp, h * D : h * D + VE],
                        rp,
                        VE,
                    )

            def finish_proj(gt, yp):
                nc.tensor.matmul(
                    yp[:, :C],
                    oTp[:, H - 1, gt * 128 : (gt + 1) * 128],
                    wpt[:, H - 1, :],
                    start=False,
                    stop=True,
                )
                acc = yacc_pool.tile([128, C], FP32, tag="acc")
                nc.vector.tensor_tensor(acc[:], yp[:, :C], bias_sb[:], ALU.add)
                nc.sync.dma_start(out_d.ap()[gt * 128 : (gt + 1) * 128, :], acc[:])

            def emit_proj_tile(gt):
                yp = s_ps.tile([128, CHUNK], FP32, tag="s")
                for h in range(H):
                    nc.tensor.matmul(
                        yp[:, :C],
                        oTp[:, h, gt * 128 : (gt + 1) * 128],
                        wpt[:, h, :],
                        start=(h == 0),
                        stop=(h == H - 1),
                    )
                acc = yacc_pool.tile([128, C], FP32, tag="acc")
                nc.vector.tensor_tensor(acc[:], yp[:, :C], bias_sb[:], ALU.add)
                nc.sync.dma_start(out_d.ap()[gt * 128 : (gt + 1) * 128, :], acc[:])

            # ---- attention ----
            from collections import deque

            jobq = deque()
            chainq = deque()
            av_backlog = deque()

            def emit_attention(h, nci, pops=1):
                n0 = nci * CHUNK
                jt = h // 2
                ot_ps = o_ps.tile([128, CHUNK], FP32, tag="ot")

                def make_av(mt, pt):
                    def f():
                        for s0 in range(0, CHUNK, 512):
                            nc.tensor.matmul(
                                ot_ps[:, s0 : s0 + 512],
                                v_sb[:, mt, h * VE : (h + 1) * VE],
                                pt[:, s0 : s0 + 512],
                                start=(mt == 0),
                                stop=(mt == MT - 1),
                            )
                    return f

                # depth-2 software pipeline: AV(mt) is emitted two iterations
                # after scores(mt), giving the scalar engine two full
                # iterations to produce exp(mt) — AV never head-of-line
                # blocks the in-order PE queue on the activation.
                for mt in range(MT):
                    if mt >= 2 and chainq:
                        chainq.popleft()()
                    for _ in range(pops):
                        if jobq:
                            jobq.popleft()()
                    sp = s_ps.tile([128, CHUNK], FP32, tag="s")
                    for s0 in range(0, CHUNK, 512):
                        nc.tensor.matmul(
                            sp[:, s0 : s0 + 512],
                            kTp[:, h, mt * 128 : (mt + 1) * 128],
                            qT[:, jt, n0 + s0 : n0 + s0 + 512],
                            start=True,
                            stop=True,
                        )
                    pt = pt_pool.tile([128, CHUNK], FP16, tag="pt")
                    nc.scalar.activation(pt[:], sp[:], AF.Exp, scale=SCALE)
                    av_backlog.append(make_av(mt, pt))
                    while len(av_backlog) > 2:
                        av_backlog.popleft()()

                # eviction + reciprocal chain, deferred into the next phase.
                # The last two AVs of this phase drain from av_backlog during
                # the next phase's first two iterations, so these jobs are
                # inserted at queue position >= 2 (after padding) to keep
                # emission order legal (they read ot_ps after AV(15)).
                box = []

                def job_evict():
                    # unscaled O^T (rows 64..127) + denominator row 0; the
                    # denominator rides row 0 of the AV output because
                    # partition_broadcast only reads physical partition 0.
                    oraw = misc.tile([128, CHUNK], FP16, tag="oraw")
                    nc.vector.tensor_copy(oraw[64:128, :], ot_ps[64:128, :])
                    dstage = misc.tile([1, CHUNK], FP32, tag="dst")
                    nc.vector.tensor_copy(dstage[0:1, :], ot_ps[0:1, :])
                    box.append(oraw)
                    box.append(dstage)

                def job_bcast():
                    denomB = misc.tile([128, CHUNK], FP32, tag="denomB")
                    nc.gpsimd.partition_broadcast(
                        denomB[:], box[1][0:1, :], channels=128
                    )
                    box.append(denomB)

                def job_recip():
                    recipB = misc.tile([128, CHUNK], FP32, tag="recipB")
                    nc.vector.reciprocal_approx_fast(recipB[:], box[2][:])
                    box.append(recipB)

                def job_mult():
                    nc.vector.tensor_tensor(
                        oTp[64:128, h, n0 : n0 + CHUNK],
                        box[0][64:128, :],
                        box[3][64:128, :],
                        ALU.mult,
                    )

                chainq.extend((job_evict, job_bcast, job_recip, job_mult))

            # job schedule.  (0,0) carries the deferred startup work (x
            # transposes for groups 2-3, k0 chunks 1-3, all v tiles) at two
            # pops per iteration; later phases spread the remaining qkv
            # projections, w_proj load, and the output projection so the PE
            # load per phase stays near the exp cadence.  k-plane chunks are
            # scheduled before the m-tiles that consume them; q-plane
            # 1024.. columns are produced before the chunk-1 phases.
            x_jobs = {mt: (lambda m=mt: emit_x_transpose(m)) for mt in range(8, MT)}
            v_job = {mt: (lambda m=mt: emit_v_tile(m)) for mt in range(MT)}
            qk_job = {(s, g): (lambda ss=s, gg=g: emit_qk_chunk(ss, gg))
                      for s in range(6) for g in range(N // 512)}
            wp_jobs = [lambda w=w: emit_wproj_tile(w) for w in range(3)]
            phase00 = [
                qk_job[3, 1], v_job[0],
                v_job[1], x_jobs[8],
                v_job[2], x_jobs[9],
                v_job[3], x_jobs[10],
                v_job[4], x_jobs[11],
                v_job[5], qk_job[3, 2],
                v_job[6], x_jobs[12],
                v_job[7], x_jobs[13],
                v_job[8], x_jobs[14],
                v_job[9], x_jobs[15],
                v_job[10], qk_job[3, 3],
            ] + [v_job[mt] for mt in range(11, MT)]
            phase_jobs = {
                (0, 0): phase00,
                (0, 1): [qk_job[1, 0], qk_job[1, 1], qk_job[4, 0],
                         qk_job[4, 1], qk_job[4, 2], qk_job[4, 3]],
                (0, 2): [qk_job[2, 0], qk_job[2, 1], qk_job[5, 0],
                         qk_job[5, 1], qk_job[5, 2], qk_job[5, 3]],
                (0, 3): wp_jobs,
                (0, 4): [qk_job[0, 2], qk_job[0, 3]],
                (1, 1): [lambda g=t: emit_proj_tile(g) for t in range(4)]
                        + [qk_job[1, 2], qk_job[1, 3]],
                (1, 2): [lambda g=t: emit_proj_tile(g) for t in range(4, PT_CH)],
                (1, 3): [qk_job[2, 2], qk_job[2, 3]],
            }
            for nci in range(NCH):
                for h in range(H):
                    jobq.extend(phase_jobs.get((nci, h), []))
                    emit_attention(h, nci, pops=2 if (nci, h) == (0, 0) else 1)
            while av_backlog:
                av_backlog.popleft()()
            while chainq:
                chainq.popleft()()
            while jobq:
                jobq.popleft()()
            # tail projection, pipelined in two stages: each tile's first
            # four head-accumulation matmuls don't depend on the final
            # head's reciprocal chain, so they run while it completes.
            pend = None
            for t in range(PT_CH):
                gt = (NCH - 1) * PT_CH + t
                yp = s_ps.tile([128, CHUNK], FP32, tag="s")
                for h in range(H - 1):
                    nc.tensor.matmul(
                        yp[:, :C],
                        oTp[:, h, gt * 128 : (gt + 1) * 128],
                        wpt[:, h, :],
                        start=(h == 0),
                        stop=False,
                    )
                if pend is not None:
                    finish_proj(*pend)
                pend = (gt, yp)
            finish_proj(*pend)

    nc.compile()
    return nc


_cache = {}


def _get_program(N: int):
    if N not in _cache:
        _cache[N] = build_program(N)
    return _cache[N]


def kernel(x, w_qkv, w_proj, b_proj):
    x = np.ascontiguousarray(np.asarray(x, dtype=np.float32))
    w_qkv = np.ascontiguousarray(np.asarray(w_qkv, dtype=np.float32))
    w_proj = np.ascontiguousarray(np.asarray(w_proj, dtype=np.float32))
    b_proj = np.ascontiguousarray(np.asarray(b_proj, dtype=np.float32))
    Bx, N, Cx = x.shape
    assert Bx == B and Cx == C, (x.shape,)

    nc = _get_program(N)
    in_maps = [
        {"x": x[b], "w_qkv": w_qkv, "w_proj": w_proj, "b_proj": b_proj}
        for b in range(B)
    ]
    res = run_bass_kernel_spmd(nc, in_maps, core_ids=list(range(B)))
    return np.stack([res.results[b]["out"] for b in range(B)], axis=0)


# revision 19
# speedup vs baseline: 1.2256x; 1.0098x over previous
"""Multi-head attention (B=8, N=2048, C=320, H=5, D=64) on 8 Trainium2 cores.

Sharding: data-parallel over batch — core b computes attention for x[b].
Weights are replicated. No collectives.

Per-core strategy (fp16 operands, fp32 accumulation):
  - Every matmul in the steady state uses the full (128,128) PE tile config
    so LDWEIGHTS always overlaps the previous matmul's streaming (a config
    switch costs ~90ns of unoverlapped weight load).  Scores contract over
    128 partitions with the per-head k^T stationary zero-padded outside the
    head's 64-channel band (kTp); the moving qT plane rows belonging to the
    sibling head are killed by the zero rows.
  - w_qkv^T is stored with each of q/k padded to 384 columns (zero columns)
    so all projection stationaries are 128 wide.
  - Scores computed transposed: S^T[m, n] = sum_d kTp[d,m] qT[d,n]; exp on
    the scalar engine produces P^T directly as the AV moving operand with
    lhsT = [V_h | ones]; row 64 of the AV output is the softmax denominator.
  - AV eviction is split so no engine queue head-of-line blocks: an
    immediate unscaled fp16 copy frees the PSUM tile, then the reciprocal
    chain (denom row -> DRAM -> [128,8] scatter -> DVE reciprocal -> row ->
    GPSIMD partition-broadcast) runs as deferred jobs inside the next
    phase, and GPSIMD (not DVE) applies the 1/denom scale into oTp.
  - The projection is a clean 5-matmul PSUM accumulation over heads plus
    one bias add; proj(chunk c) is interleaved two phases after the last
    head of chunk c so the reciprocal chain is never on the critical path.
  - PSUM: scores pool 2x[128,1024] + output pool 2x[65,1024] = 8 banks; the
    double-buffered output tile removes the per-head eviction stall.
  - fp32->fp16 input casts run on the scalar engine (idle during startup);
    startup PE transposes rotate across both PSUM pools.
"""

import numpy as np

import concourse.bacc as bacc
import concourse.tile as tile
from concourse import mybir
from concourse.bass_utils import run_bass_kernel_spmd
from concourse.masks import make_identity

FP32 = mybir.dt.float32
FP16 = mybir.dt.float16
AF = mybir.ActivationFunctionType
ALU = mybir.AluOpType

B = 8
C = 320
H = 5
D = 64
SCALE = D ** -0.5
# c-dim tiles (contraction tiles for the C=320 dim); all matmuls use the
# full 128 partitions — rows 64..127 of the third tile are zeroed.
CT = [(0, 128), (128, 128), (256, 64)]
# padded w^T column sections: q @ 0, k @ 384, v @ 768 (each q/k padded to 384)
QSEC, KSEC, VSEC = 0, 384, 768
WCOLS = 1088


def build_program(N: int):
    """Build + compile the single-core Bass program (SPMD across 8 cores)."""
    nc = bacc.Bacc("TRN2", target_bir_lowering=False, debug=False)

    x_d = nc.dram_tensor("x", [N, C], FP32, kind="ExternalInput")
    wqkv_d = nc.dram_tensor("w_qkv", [3 * C, C], FP32, kind="ExternalInput")
    wproj_d = nc.dram_tensor("w_proj", [C, C], FP32, kind="ExternalInput")
    bproj_d = nc.dram_tensor("b_proj", [C], FP32, kind="ExternalInput")
    out_d = nc.dram_tensor("out", [N, C], FP32, kind="ExternalOutput")

    MT = N // 128                       # number of 128-row seq tiles
    CHUNK = 1024 if N % 1024 == 0 else N
    NCH = N // CHUNK                    # attention n-chunks
    PT_CH = CHUNK // 128                # proj n-tiles per chunk

    with tile.TileContext(nc) as tc:
        with (
            tc.tile_pool(name="persist", bufs=1) as per,
            tc.tile_pool(name="ld", bufs=3) as ld,
            tc.tile_pool(name="s_ps", bufs=2, space="PSUM") as s_ps,
            tc.tile_pool(name="o_ps", bufs=2, space="PSUM") as o_ps,
            tc.tile_pool(name="pt", bufs=4) as pt_pool,
            tc.tile_pool(name="misc", bufs=2) as misc,
            tc.tile_pool(name="yacc", bufs=3) as yacc_pool,
        ):
            identity = per.tile([128, 128], FP32)
            make_identity(nc, identity[:])
            identity_h = per.tile([128, 128], FP16)
            nc.vector.tensor_copy(identity_h[:], identity[:])
            # warm the PE p-state while the first DMAs are in flight: the
            # clock ramps to max only after ~3us of continuous execution.
            for _ in range(10):
                wps = s_ps.tile([128, 512], FP16, tag="s")
                nc.tensor.transpose(wps[:128, :128], identity_h[:], identity_h[:])

            wT = per.tile([128, 3, WCOLS], FP16)   # w_qkv^T, padded sections
            xT = per.tile([128, 3, N], FP16)
            qT = per.tile([128, 3, N], FP16)
            kTp = per.tile([128, H, N], FP16)      # per-head k^T, zero-padded
            VE = 128  # per-head AV stationary: [ones | 63 pad | V(64)]
            v_sb = per.tile([128, MT, H * VE], FP16)
            oTp = per.tile([128, H, N], FP16)      # rows 0-63: O^T_h scaled
            wpt = per.tile([128, H, C], FP16)      # w_proj^T per head
            b_row = per.tile([1, C], FP16)

            # ---- zero padding memsets ----
            # kTp band for head h occupies partitions 64*(h%2)..; the
            # complement must be zero (it is the scores stationary).
            v_heads = v_sb[:].rearrange("p m (h e) -> p m h e", h=H)
            nc.gpsimd.memset(v_heads[:, :, :, 0:1], 1.0)
            nc.gpsimd.memset(v_heads[:, :, :, 1:64], 0.0)
            nc.gpsimd.memset(xT[64:128, 2, :], 0.0)
            nc.gpsimd.memset(wT[64:128, 2, :], 0.0)
            for h in range(2):
                off = 64 * (h % 2)
                nc.gpsimd.memset(kTp[64 - off : 128 - off, h, :], 0.0)
            nc.vector.memset(wT[:, :, QSEC + 320 : QSEC + 384], 0.0)
            nc.vector.memset(wT[:, :, KSEC + 320 : KSEC + 384], 0.0)
            for h in range(2, H):
                off = 64 * (h % 2)
                nc.gpsimd.memset(kTp[64 - off : 128 - off, h, :], 0.0)
            nc.gpsimd.memset(oTp[0:64, :, :], 0.0)
            nc.vector.memset(oTp[0:1, H - 1, :], 1.0)

            # startup transposes rotate across both PSUM pools (o_ps is idle
            # until the first attention phase)
            tp_state = [0]

            def transpose_fp16(dst_ap, src_ap, rp, cp, startup=False):
                """dst[cp, rp] = src[rp, cp].T via PE transpose (fp16)."""
                if startup and tp_state[0] % 2:
                    ps = o_ps.tile([128, 512], FP16, tag="ot")
                else:
                    ps = s_ps.tile([128, 512], FP16, tag="s")
                tp_state[0] += 1
                nc.tensor.transpose(ps[:cp, :rp], src_ap, identity_h[:rp, :rp])
                nc.vector.tensor_copy(dst_ap, ps[:cp, :rp])

            # ---- w_qkv -> wT (w_qkv^T into padded sections) ----
            def wcol(r):
                """padded wT column for w_qkv row r."""
                if r < 320:
                    return QSEC + r
                if r < 640:
                    return KSEC + (r - 320)
                return VSEC + (r - 640)

            def emit_w_tile(wt, startup=True):
                r0 = wt * 128
                rp = min(128, 3 * C - r0)
                wnat = ld.tile([128, C], FP32, tag="wnat")
                nc.sync.dma_start(wnat[:rp, :], wqkv_d.ap()[r0 : r0 + rp, :])
                wnat_h = ld.tile([128, C], FP16, tag="wnat_h")
                nc.scalar.activation(wnat_h[:rp, :], wnat[:rp, :], AF.Copy)
                splits = [r0]
                for bnd in (320, 640):
                    if r0 < bnd < r0 + rp:
                        splits.append(bnd)
                splits.append(r0 + rp)
                for ci, (c0, cp) in enumerate(CT):
                    if startup and tp_state[0] % 2:
                        ps = o_ps.tile([128, 512], FP16, tag="ot")
                    else:
                        ps = s_ps.tile([128, 512], FP16, tag="s")
                    tp_state[0] += 1
                    nc.tensor.transpose(
                        ps[:cp, :rp],
                        wnat_h[:rp, c0 : c0 + cp],
                        identity_h[:rp, :rp],
                    )
                    for a, b_ in zip(splits, splits[1:]):
                        d0 = wcol(a)
                        nc.vector.tensor_copy(
                            wT[:cp, ci, d0 : d0 + (b_ - a)],
                            ps[:cp, a - r0 : b_ - r0],
                        )

            # ---- x -> xT (DMA, ACT cast, PE transpose) + q0/k0 chunks ----
            # qk section s (0..5 = q0,q1,q2p,k0,k1,k2p): 128 padded channels.
            def emit_qk_chunk(sec, g):
                base = (QSEC if sec < 3 else KSEC) + 128 * (sec % 3)
                s0 = g * 512
                sw = min(512, N - s0)
                ps = s_ps.tile([128, 512], FP32, tag="s")
                for ci in range(3):
                    nc.tensor.matmul(
                        ps[:, :sw],
                        wT[:, ci, base : base + 128],
                        xT[:, ci, s0 : s0 + sw],
                        start=(ci == 0),
                        stop=(ci == 2),
                    )
                if sec < 3:
                    # q plane j=sec; full 128 rows (zero rows land in plane 2)
                    nc.vector.tensor_copy(qT[:, sec, s0 : s0 + sw], ps[:, :sw])
                else:
                    j = sec - 3
                    h0 = 2 * j
                    nc.vector.tensor_copy(
                        kTp[0:64, h0, s0 : s0 + sw], ps[0:64, :sw]
                    )
                    if h0 + 1 < H:
                        nc.vector.tensor_copy(
                            kTp[64:128, h0 + 1, s0 : s0 + sw], ps[64:128, :sw]
                        )

            # DMA + cast ALL x groups up front (4-deep ld buffers: the DMA
            # stream never waits on recycling); per-group casts run on the
            # scalar engine which is idle before the first exp.
            x_re = x_d.ap().rearrange("(t p) c -> p t c", p=128)
            NG = (MT + 3) // 4
            xh_tiles = []

            def emit_x_group(g):
                gn = min(4, MT - 4 * g)
                xnat = ld.tile([128, 4, C], FP32, tag="xnat", bufs=NG)
                nc.sync.dma_start(xnat[:, :gn, :], x_re[:, 4 * g : 4 * g + gn, :])
                xnat_h = ld.tile([128, 4, C], FP16, tag="xnat_h", bufs=NG)
                nc.scalar.activation(xnat_h[:, :gn, :], xnat[:, :gn, :], AF.Copy)
                xh_tiles.append(xnat_h)

            # interleave the w and x loads so neither the transposes (need
            # x early) nor the q0/k0 projections (need w0/2/3) starve.
            emit_w_tile(0)
            emit_x_group(0)
            emit_w_tile(2)
            emit_w_tile(3)
            emit_x_group(1)

            def emit_x_transpose(mt):
                xnat_h = xh_tiles[mt // 4]
                t = mt % 4
                for ci, (c0, cp) in enumerate(CT):
                    transpose_fp16(
                        xT[:cp, ci, mt * 128 : (mt + 1) * 128],
                        xnat_h[:, t, c0 : c0 + cp],
                        128,
                        cp,
                        startup=True,
                    )

            # prologue PE work: only what the first scores iteration needs —
            # x groups 0-1, q0 over cols 0-1023, k0 over m-tiles 0-3, and the
            # v section weights (v tiles start at the first attention
            # iteration).  The rest defers into attention phases as jobs.
            for mt in range(4):
                emit_x_transpose(mt)
            emit_x_group(2)
            emit_w_tile(5)
            for mt in range(4, 8):
                emit_x_transpose(mt)
            emit_x_group(3)
            emit_w_tile(6)
            emit_w_tile(7)
            emit_qk_chunk(0, 0)
            emit_qk_chunk(0, 1)
            emit_qk_chunk(3, 0)
            emit_w_tile(4)
            emit_w_tile(1)

            # ---- bias row (lands in wpt[0, H-1, :] after the wproj
            # transposes; oTp row 0 of head H-1 is 1.0, so the projection
            # accumulation adds the bias for free) ----
            b32 = per.tile([1, C], FP32)
            nc.sync.dma_start(b32[:], bproj_d.ap().rearrange("(a c) -> a c", a=1))
            nc.vector.tensor_copy(b_row[:], b32[:])

            # ---- interleavable jobs ----
            def emit_v_tile(mt):
                ps = s_ps.tile([128, 512], FP32, tag="s")
                for ci in range(3):
                    nc.tensor.matmul(
                        ps[:, :C],
                        xT[:, ci, mt * 128 : (mt + 1) * 128],
                        wT[:, ci, VSEC : VSEC + C],
                        start=(ci == 0),
                        stop=(ci == 2),
                    )
                nc.vector.tensor_copy(
                    v_heads[:, mt, :, 64 : 64 + D],
                    ps[:, :C].rearrange("p (h e) -> p h e", h=H),
                )

            def emit_wproj_tile(wt):
                r0, rp = CT[wt]
                wpnat = ld.tile([128, C], FP32, tag="wnat")
                nc.sync.dma_start(wpnat[:rp, :], wproj_d.ap()[r0 : r0 + rp, :])
                # 64 pad columns in front: transposing [hD-64 .. hD+63]
                # puts w_proj channel d at wpt row 64+d, matching the AV
                # output rows (rows <64 hit pad/other-head junk, which the
                # zero rows 0..63 of oTp kill in the proj matmul).
                wpnat_h = ld.tile([128, 64 + C], FP16, tag="wpnat_h")
                nc.vector.memset(wpnat_h[:, 0:64], 0.0)
                nc.scalar.activation(
                    wpnat_h[:rp, 64 : 64 + C], wpnat[:rp, :], AF.Copy
                )
                for h in range(H):
                    transpose_fp16(
                        wpt[0:VE, h, r0 : r0 + rp],
                        wpnat_h[:rp, h * D : h * D + VE],
                        rp,
                        VE,
                    )

            def finish_proj(gt, yp):
                nc.tensor.matmul(
                    yp[:, :C],
                    oTp[:, H - 1, gt * 128 : (gt + 1) * 128],
                    wpt[:, H - 1, :],
                    start=False,
                    stop=True,
                )
                acc = yacc_pool.tile([128, C], FP32, tag="acc")
                nc.vector.tensor_copy(acc[:], yp[:, :C])
                nc.sync.dma_start(out_d.ap()[gt * 128 : (gt + 1) * 128, :], acc[:])

            def emit_proj_tile(gt):
                yp = s_ps.tile([128, CHUNK], FP32, tag="s")
                for h in range(H):
                    nc.tensor.matmul(
                        yp[:, :C],
                        oTp[:, h, gt * 128 : (gt + 1) * 128],
                        wpt[:, h, :],
                        start=(h == 0),
                        stop=(h == H - 1),
                    )
                acc = yacc_pool.tile([128, C], FP32, tag="acc")
                nc.vector.tensor_copy(acc[:], yp[:, :C])
                nc.sync.dma_start(out_d.ap()[gt * 128 : (gt + 1) * 128, :], acc[:])

            # ---- attention ----
            from collections import deque

            jobq = deque()
            chainq = deque()
            av_backlog = deque()

            def emit_attention(h, nci, pops=1, last=False):
                n0 = nci * CHUNK
                jt = h // 2
                ot_ps = o_ps.tile([128, CHUNK], FP32, tag="ot")

                def make_av(mt, pt):
                    def f():
                        for s0 in range(0, CHUNK, 512):
                            nc.tensor.matmul(
                                ot_ps[:, s0 : s0 + 512],
                                v_sb[:, mt, h * VE : (h + 1) * VE],
                                pt[:, s0 : s0 + 512],
                                start=(mt == 0),
                                stop=(mt == MT - 1),
                            )
                    return f

                # depth-2 software pipeline: AV(mt) is emitted two iterations
                # after scores(mt), giving the scalar engine two full
                # iterations to produce exp(mt) — AV never head-of-line
                # blocks the in-order PE queue on the activation.
                for mt in range(MT):
                    if mt >= 2 and chainq:
                        chainq.popleft()()
                    for _ in range(pops):
                        if jobq:
                            jobq.popleft()()
                    sp = s_ps.tile([128, CHUNK], FP32, tag="s")
                    for s0 in range(0, CHUNK, 512):
                        nc.tensor.matmul(
                            sp[:, s0 : s0 + 512],
                            kTp[:, h, mt * 128 : (mt + 1) * 128],
                            qT[:, jt, n0 + s0 : n0 + s0 + 512],
                            start=True,
                            stop=True,
                        )
                    pt = pt_pool.tile([128, CHUNK], FP16, tag="pt")
                    nc.scalar.activation(pt[:], sp[:], AF.Exp, scale=SCALE)
                    av_backlog.append(make_av(mt, pt))
                    while len(av_backlog) > 2:
                        av_backlog.popleft()()

                # eviction + reciprocal chain, deferred into the next phase.
                # The last two AVs of this phase drain from av_backlog during
                # the next phase's first two iterations, so these jobs are
                # inserted at queue position >= 2 (after padding) to keep
                # emission order legal (they read ot_ps after AV(15)).
                box = []

                def job_evict():
                    # unscaled O^T (rows 64..127) + denominator row 0; the
                    # denominator rides row 0 of the AV output because
                    # partition_broadcast only reads physical partition 0.
                    oraw = misc.tile([128, CHUNK], FP16, tag="oraw")
                    nc.vector.tensor_copy(oraw[64:128, :], ot_ps[64:128, :])
                    dstage = misc.tile([1, CHUNK], FP32, tag="dst")
                    nc.vector.tensor_copy(dstage[0:1, :], ot_ps[0:1, :])
                    box.append(oraw)
                    box.append(dstage)

                def job_bcast():
                    denomB = misc.tile([128, CHUNK], FP32, tag="denomB")
                    nc.gpsimd.partition_broadcast(
                        denomB[:], box[1][0:1, :], channels=128
                    )
                    box.append(denomB)

                def job_recip():
                    recipB = misc.tile([128, CHUNK], FP32, tag="recipB")
                    nc.vector.reciprocal_approx_fast(recipB[:], box[2][:])
                    box.append(recipB)

                def job_mult():
                    nc.vector.tensor_tensor(
                        oTp[64:128, h, n0 : n0 + CHUNK],
                        box[0][64:128, :],
                        box[3][64:128, :],
                        ALU.mult,
                    )

                if last:
                    # nothing follows: drain the AV backlog and run the
                    # reciprocal chain immediately so the tail projection
                    # starts as early as possible.
                    while av_backlog:
                        av_backlog.popleft()()
                    for job in (job_evict, job_bcast, job_recip, job_mult):
                        job()
                else:
                    chainq.extend((job_evict, job_bcast, job_recip, job_mult))

            # job schedule.  (0,0) carries the deferred startup work (x
            # transposes for groups 2-3, k0 chunks 1-3, all v tiles) at two
            # pops per iteration; later phases spread the remaining qkv
            # projections, w_proj load, and the output projection so the PE
            # load per phase stays near the exp cadence.  k-plane chunks are
            # scheduled before the m-tiles that consume them; q-plane
            # 1024.. columns are produced before the chunk-1 phases.
            x_jobs = {mt: (lambda m=mt: emit_x_transpose(m)) for mt in range(8, MT)}
            v_job = {mt: (lambda m=mt: emit_v_tile(m)) for mt in range(MT)}
            qk_job = {(s, g): (lambda ss=s, gg=g: emit_qk_chunk(ss, gg))
                      for s in range(6) for g in range(N // 512)}
            wp_jobs = [lambda w=w: emit_wproj_tile(w) for w in range(3)]
            phase00 = [
                qk_job[3, 1], v_job[0],
                v_job[1], x_jobs[8],
                v_job[2], x_jobs[9],
                v_job[3], x_jobs[10],
                v_job[4], x_jobs[11],
                v_job[5], qk_job[3, 2],
                v_job[6], x_jobs[12],
                v_job[7], x_jobs[13],
                v_job[8], x_jobs[14],
                v_job[9], x_jobs[15],
                v_job[10], qk_job[3, 3],
            ] + [v_job[mt] for mt in range(11, MT)]
            def emit_bias_row():
                nc.vector.tensor_copy(wpt[0:1, H - 1, :], b_row[:])

            phase_jobs = {
                (0, 0): phase00,
                (0, 1): [qk_job[1, 0], qk_job[1, 1], qk_job[4, 0],
                         qk_job[4, 1], qk_job[4, 2], qk_job[4, 3]],
                (0, 2): [qk_job[2, 0], qk_job[2, 1], qk_job[5, 0],
                         qk_job[5, 1], qk_job[5, 2], qk_job[5, 3]],
                (0, 3): wp_jobs,
                (0, 4): [qk_job[0, 2], qk_job[0, 3], emit_bias_row],
                (1, 1): [lambda g=t: emit_proj_tile(g) for t in range(4)]
                        + [qk_job[1, 2], qk_job[1, 3]],
                (1, 2): [lambda g=t: emit_proj_tile(g) for t in range(4, PT_CH)],
                (1, 3): [qk_job[2, 2], qk_job[2, 3]],
            }
            for nci in range(NCH):
                for h in range(H):
                    jobq.extend(phase_jobs.get((nci, h), []))
                    emit_attention(
                        h, nci,
                        pops=2 if (nci, h) == (0, 0) else 1,
                        last=(nci, h) == (NCH - 1, H - 1),
                    )
            while av_backlog:
                av_backlog.popleft()()
            while chainq:
                chainq.popleft()()
            while jobq:
                jobq.popleft()()
            # tail projection, pipelined in two stages: each tile's first
            # four head-accumulation matmuls don't depend on the final
            # head's reciprocal chain, so they run while it completes.
            pend = None
            for t in range(PT_CH):
                gt = (NCH - 1) * PT_CH + t
                if t % 2:
                    yp = o_ps.tile([128, CHUNK], FP32, tag="ot")
                else:
                    yp = s_ps.tile([128, CHUNK], FP32, tag="s")
                for h in range(H - 1):
                    nc.tensor.matmul(
                        yp[:, :C],
                        oTp[:, h, gt * 128 : (gt + 1) * 128],
                        wpt[:, h, :],
                        start=(h == 0),
                        stop=False,
                    )
                if pend is not None:
                    finish_proj(*pend)
                pend = (gt, yp)
            finish_proj(*pend)

    nc.compile()
    return nc


_cache = {}


def _get_program(N: int):
    if N not in _cache:
        _cache[N] = build_program(N)
    return _cache[N]


def kernel(x, w_qkv, w_proj, b_proj):
    x = np.ascontiguousarray(np.asarray(x, dtype=np.float32))
    w_qkv = np.ascontiguousarray(np.asarray(w_qkv, dtype=np.float32))
    w_proj = np.ascontiguousarray(np.asarray(w_proj, dtype=np.float32))
    b_proj = np.ascontiguousarray(np.asarray(b_proj, dtype=np.float32))
    Bx, N, Cx = x.shape
    assert Bx == B and Cx == C, (x.shape,)

    nc = _get_program(N)
    in_maps = [
        {"x": x[b], "w_qkv": w_qkv, "w_proj": w_proj, "b_proj": b_proj}
        for b in range(B)
    ]
    res = run_bass_kernel_spmd(nc, in_maps, core_ids=list(range(B)))
    return np.stack([res.results[b]["out"] for b in range(B)], axis=0)
